# revision 20
# baseline (speedup 1.0000x reference)
"""Trainium2 Bass kernel for nn_GatedLinear (gated LoRA-MoE linear layer).

Math (see reference):
  base_out = x @ base_w.T + base_b
  logits   = x @ router_w.T ; top-2 softmax -> dense per-expert gate
  h        = x @ lora_A.T   ; rank_w = repeat(gate*scalings, 16)
  out      = base_out + (h * rank_w) @ lora_B.T

Sharding: pure data-parallel over batch*seq across 8 cores (1024 tokens
per core); all weights replicated. No collectives.

Device-side strategy (v5):
  * The host ships three copies of x.T: exact fp32 (router only --
    top-2 selection must match the fp32 reference bit-for-bit), bf16
    (h matmul), and fp8e4m3 (base matmul). fp32 chunks go through a
    small ring on two HWDGE queues; bf16/fp8 chunks DMA straight into
    resident tiles on the gpsimd queue. Sub-tile deps let every
    consumer start as soon as its ko-chunk lands.
  * Base matmul runs fp8e4m3 with perf_mode=DoubleRow: weights are
    host-scaled x64 into e4m3's range and packed [k2, 2, f] so each
    matmul contracts 256 deep -- half the instructions of bf16, 216ns
    per 256x128x512 step. The x64 scale is folded into the e8 gate
    expansion (so the lora step accumulates at the same scale) and
    removed in the bias epilogue (acc/64 + b, one DVE op).
  * lora_B.T stays resident bf16 and closes each PSUM accumulation
    group as a 33rd step with the gated rank activations; lora_A/h and
    the gating chain stay bf16/fp32, keeping total rel err ~1.1e-2
    (gate is 2e-2) with the fp8 noise confined to the base term.
  * Router matmuls (exact fp32) and h matmuls interleave across the
    ko-chunks so the PE stays busy while x streams in; gating runs
    token-major (PE transposes + DVE top-2 chain).
  * Weight DRAM layout is [ot, p, (k2 j f)] so each per-ot weight DMA
    is 128 contiguous runs (fast descriptor gen + full DMA bw).
  * DMA queues: fp32 x alternates sync/scalar; bf16+fp8 x and lora
    consts on gpsimd; weight stream on scalar; outputs on sync.

Output is produced transposed ([out_features, tokens] per core) and
de-transposed on the host.
"""

from contextlib import ExitStack

import numpy as np


def _ensure_path():
    try:
        import concourse.bass  # noqa: F401
    except ImportError:
        import sys

        for p in ("/opt/trn_rl_repo", "/root/.axon_site/_ro/trn_rl_repo"):
            if p not in sys.path:
                sys.path.insert(0, p)


N_CORES = 8
B, S, D, O = 4, 2048, 4096, 4096
T = B * S              # 8192 tokens total
T_PC = T // N_CORES    # 1024 tokens per core
E = 8                  # experts
RANK = 16
R = E * RANK           # 128 fused rank dim
P = 128
KO = D // P            # 32 k-subtiles of the contraction dim
KO_EXT = KO + 1        # +1 subtile holding lora_B.T
OTILES = O // P        # 32 output-feature tiles
TTILE = 512            # tokens per matmul moving operand
NT = T_PC // TTILE     # 2 token tiles per core
GT = 256               # gating token-tile size
NGT = T_PC // GT       # 4 gating tiles
NGC = GT // P          # 128-chunks per gating tile

WT_BF16 = True         # bf16 stationary weights (mixed with f32r moving)
FP8_BASE = True        # fp8e4m3 DoubleRow for the base matmul (2x PE rate)
KO2 = KO // 2          # paired k-subtiles for DoubleRow (256-deep contraction)
W8_SCALE = 64.0        # base_w std is 1/64; scale into e4m3's sweet spot

_prog_cache = {}


def _build_program():
    """Build the single-core SPMD Bass program (same on all 8 cores)."""
    _ensure_path()
    import concourse.bass as bass
    import concourse.mybir as mybir
    import concourse.tile as tile
    from concourse import bacc

    f32 = mybir.dt.float32
    f32r = mybir.dt.float32r
    bf16 = mybir.dt.bfloat16
    f8 = mybir.dt.float8e4
    wdt = bf16 if WT_BF16 else f32r
    Alu = mybir.AluOpType
    Act = mybir.ActivationFunctionType
    DR = mybir.MatmulPerfMode.DoubleRow

    nc = bacc.Bacc(
        "TRN2",
        target_bir_lowering=False,
        debug=False,
        num_devices=N_CORES,
    )

    xt = nc.dram_tensor("xt", [D, T_PC], f32, kind="ExternalInput").ap()
    xb = nc.dram_tensor("xb", [D, T_PC], bf16, kind="ExternalInput").ap()
    x8d = nc.dram_tensor("x8", [D, T_PC], f8, kind="ExternalInput").ap()
    xb_v = xb.rearrange("(ko p) t -> p ko t", p=P)
    x8_v = x8d.rearrange("(ko p) t -> p ko t", p=P)
    if FP8_BASE:
        # base weights only, x64-scaled fp8, DoubleRow pair layout
        wt = nc.dram_tensor(
            "wt", [OTILES * P, KO2 * 2 * P], f8, kind="ExternalInput"
        ).ap()
        wt_v = wt.rearrange("(ot p) (k j f) -> p ot k j f", p=P, j=2, f=P)
        lb = nc.dram_tensor("lb", [P, O], wdt, kind="ExternalInput").ap()
    else:
        wt = nc.dram_tensor(
            "wt", [OTILES * P, KO_EXT * P], wdt, kind="ExternalInput"
        ).ap()
        wt_v = wt.rearrange("(ot p) (ko f) -> p ot ko f", p=P, f=P)
        lb = None
    ar = nc.dram_tensor("ar", [P, KO * R], wdt, kind="ExternalInput").ap()
    rt = nc.dram_tensor("rt", [P, KO * E], f32, kind="ExternalInput").ap()
    bb = nc.dram_tensor("bb", [O], f32, kind="ExternalInput").ap()
    e8 = nc.dram_tensor("e8", [E, P], f32, kind="ExternalInput").ap()
    idm = nc.dram_tensor("idm", [P, P], f32, kind="ExternalInput").ap()
    yt = nc.dram_tensor("yt", [O, T_PC], f32, kind="ExternalOutput").ap()

    xt_v = xt.rearrange("(ko p) t -> p ko t", p=P)        # [128, 32, 1024]
    ar_v = ar.rearrange("p (ko r) -> p ko r", r=R)        # [128, 32, 128]
    rt_v = rt.rearrange("p (ko e) -> p ko e", e=E)        # [128, 32, 8]
    bb_v = bb.rearrange("(ot p) -> p ot", p=P)            # [128, 32]
    yt_v = yt.rearrange("(ot p) t -> p ot t", p=P)        # [128, 32, 1024]

    with tile.TileContext(nc) as tc:
        with (
            tc.tile_pool(name="perm", bufs=1) as pp,
            tc.tile_pool(name="obuf", bufs=3) as ob,
        ):
            # ---- small permanent constants (scalar/Activation queue) ----
            rsbf = pp.tile([P, KO, E], f32)
            nc.scalar.dma_start(rsbf[:], rt_v[:])
            bbsb = pp.tile([P, OTILES], f32)
            nc.scalar.dma_start(bbsb[:], bb_v[:])
            e8sb = pp.tile([E, P], f32)
            nc.scalar.dma_start(e8sb[:], e8[:])
            idsb = pp.tile([P, P], f32)
            nc.scalar.dma_start(idsb[:], idm[:])

            rgp = pp.tile([P, T_PC], f32)    # per-rank gates [r, t]
            hwsb = pp.tile([P, T_PC], wdt)   # gated rank activations [r, t]

            # resident bf16 copy of x (for h), filled chunk-by-chunk
            xsb = pp.tile([P, KO, T_PC], wdt)
            if FP8_BASE:
                # resident fp8 copy of x (for the base matmul)
                x8sb = pp.tile([P, KO, T_PC], f8)
                # lora_B.T resident bf16 (stationary of the lora matmul)
                lbsb = pp.tile([P, O], wdt)
                nc.gpsimd.dma_start(lbsb[:], lb[:])

            # lora_A.T (bf16 stationary), gpsimd queue
            arsb = pp.tile([P, KO, R], wdt)
            nc.gpsimd.dma_start(arsb[:], ar_v[:])

            # ---- phase 1: stream x; router logits (exact fp32) ----
            phase1 = ExitStack()
            xfp = phase1.enter_context(tc.tile_pool(name="xring", bufs=6))
            gp = phase1.enter_context(tc.tile_pool(name="gtmp", bufs=1))
            ps_l = phase1.enter_context(
                tc.tile_pool(name="ps_l", bufs=NGT, space="PSUM")
            )
            ps_b = phase1.enter_context(
                tc.tile_pool(name="ps_b", bufs=2, space="PSUM")
            )
            ps_h = phase1.enter_context(
                tc.tile_pool(name="ps_h", bufs=2, space="PSUM")
            )

            # interleave the 4 gating tiles' router matmuls and the h
            # matmuls per ko-chunk so the PE starts as soon as the first
            # x chunk lands; x alternates two DMA queues for bandwidth
            lgs_t = [
                ps_l.tile([E, GT], f32, tag="lg", name=f"lg{g}")
                for g in range(NGT)
            ]
            h_t = [
                ps_h.tile([P, TTILE], f32, tag="h", name=f"h{t}")
                for t in range(NT)
            ]
            for ko in range(KO):
                xc = xfp.tile([P, T_PC], f32, tag="xc")
                dma_q = nc.sync if ko % 2 == 0 else nc.scalar
                dma_q.dma_start(xc[:], xt_v[:, ko, :])
                nc.gpsimd.dma_start(xsb[:, ko, :], xb_v[:, ko, :])
                if FP8_BASE:
                    nc.gpsimd.dma_start(x8sb[:, ko, :], x8_v[:, ko, :])
                for g in range(NGT):
                    gs = slice(g * GT, (g + 1) * GT)
                    nc.tensor.matmul(
                        lgs_t[g][:],
                        lhsT=rsbf[:, ko, :],
                        rhs=xc[:, gs],
                        start=(ko == 0),
                        stop=(ko == KO - 1),
                    )
                for tt in range(NT):
                    ts = slice(tt * TTILE, (tt + 1) * TTILE)
                    nc.tensor.matmul(
                        h_t[tt][:],
                        lhsT=arsb[:, ko, :],
                        rhs=xsb[:, ko, ts],
                        start=(ko == 0),
                        stop=(ko == KO - 1),
                    )

            for g in range(NGT):
                gs = slice(g * GT, (g + 1) * GT)
                lgs = gp.tile([E, GT], f32, tag="lgs")
                nc.vector.tensor_copy(lgs[:], lgs_t[g][:])

                # transpose logits to token-major: [tok, chunk, expert]
                ltk = gp.tile([P, NGC, E], f32, tag="ltk")
                for c in range(NGC):
                    tp = ps_b.tile([P, GT], f32, tag="pb", name="tp")[:, :E]
                    nc.tensor.transpose(
                        tp[:], lgs[:, c * P : (c + 1) * P], idsb[:E, :E]
                    )
                    nc.vector.tensor_copy(ltk[:, c, :], tp[:])

                # top-2 + softmax along the free (expert) axis.
                m1 = gp.tile([P, NGC, 1], f32, tag="m1")
                nc.vector.tensor_reduce(m1[:], ltk[:], mybir.AxisListType.X, Alu.max)
                mask1 = gp.tile([P, NGC, E], f32, tag="mask1")
                nc.vector.tensor_tensor(
                    mask1[:], ltk[:], m1.to_broadcast((P, NGC, E)), Alu.is_equal
                )
                l2 = gp.tile([P, NGC, E], f32, tag="l2")
                nc.vector.scalar_tensor_tensor(
                    l2[:], mask1[:], -1e30, ltk[:], Alu.mult, Alu.add
                )
                m2 = gp.tile([P, NGC, 1], f32, tag="m2")
                nc.vector.tensor_reduce(m2[:], l2[:], mybir.AxisListType.X, Alu.max)
                mask2 = gp.tile([P, NGC, E], f32, tag="mask2")
                nc.vector.tensor_tensor(
                    mask2[:], l2[:], m2.to_broadcast((P, NGC, E)), Alu.is_equal
                )
                dlt = gp.tile([P, NGC, 1], f32, tag="dlt")
                nc.vector.tensor_tensor(dlt[:], m2[:], m1[:], Alu.subtract)
                g2 = gp.tile([P, NGC, 1], f32, tag="g2")
                nc.scalar.activation(g2[:], dlt[:], Act.Sigmoid)
                g1 = gp.tile([P, NGC, 1], f32, tag="g1")
                nc.vector.tensor_scalar(g1[:], g2[:], -1.0, 1.0, Alu.mult, Alu.add)

                gate = gp.tile([P, NGC, E], f32, tag="gate")
                nc.vector.tensor_tensor(
                    gate[:], mask1[:], g1.to_broadcast((P, NGC, E)), Alu.mult
                )
                gm2 = gp.tile([P, NGC, E], f32, tag="gm2")
                nc.vector.tensor_tensor(
                    gm2[:], mask2[:], g2.to_broadcast((P, NGC, E)), Alu.mult
                )
                nc.vector.tensor_tensor(gate[:], gate[:], gm2[:], Alu.add)

                # transpose gates back to expert-major [8, 256]
                gts = gp.tile([E, GT], f32, tag="gts")
                for c in range(NGC):
                    tp2 = ps_b.tile([P, GT], f32, tag="pb", name="tp2")[:E, :P]
                    nc.tensor.transpose(tp2[:], gate[:, c, :], idsb[:])
                    nc.vector.tensor_copy(gts[:, c * P : (c + 1) * P], tp2[:])

                # expand expert gates (x scaling, folded into e8) to the
                # 128 rank slots: RG = e8.T @ gts
                RG = ps_b.tile([P, GT], f32, tag="pb", name="RG")
                nc.tensor.matmul(
                    RG[:], lhsT=e8sb[:], rhs=gts[:], start=True, stop=True
                )
                nc.vector.tensor_copy(rgp[:, gs], RG[:])

            # gated rank activations (x64-scaled when FP8_BASE: the x64 is
            # folded into e8 -> rgp, cancelling the fp8 weight scale)
            for tt in range(NT):
                ts = slice(tt * TTILE, (tt + 1) * TTILE)
                nc.vector.tensor_tensor(
                    hwsb[:, ts], h_t[tt][:], rgp[:, ts], Alu.mult
                )

            phase1.close()

            # ---- phase 2: base matmul + fused lora_B ----
            phase2 = ExitStack()
            ps_o = phase2.enter_context(
                tc.tile_pool(name="ps_o", bufs=5, space="PSUM")
            )
            wpool = phase2.enter_context(tc.tile_pool(name="wstream", bufs=3))

            for ot in range(OTILES):
                os_ = slice(ot * P, (ot + 1) * P)
                if FP8_BASE:
                    wsb = wpool.tile([P, KO2, 2, P], f8, tag="w")
                    nc.scalar.dma_start(wsb[:], wt_v[:, ot, :, :, :])
                    for tt in range(NT):
                        ts = slice(tt * TTILE, (tt + 1) * TTILE)
                        acc = ps_o.tile([P, TTILE], f32, tag="acc")
                        for k2 in range(KO2):
                            nc.tensor.matmul(
                                acc[:],
                                lhsT=wsb[:, k2, :, :],
                                rhs=x8sb[:, 2 * k2 : 2 * k2 + 2, ts],
                                start=(k2 == 0),
                                stop=False,
                                perf_mode=DR,
                            )
                        nc.tensor.matmul(
                            acc[:],
                            lhsT=lbsb[:, os_],
                            rhs=hwsb[:, ts],
                            start=False,
                            stop=True,
                        )
                        osb = ob.tile([P, TTILE], f32, tag="osb")
                        # acc holds 64x(base+lora); rescale + bias in one op
                        nc.vector.scalar_tensor_tensor(
                            osb[:],
                            acc[:],
                            1.0 / W8_SCALE,
                            bbsb[:, ot, None].to_broadcast((P, TTILE)),
                            Alu.mult,
                            Alu.add,
                        )
                        nc.sync.dma_start(yt_v[:, ot, ts], osb[:])
                else:
                    wsb = wpool.tile([P, KO_EXT, P], wdt, tag="w")
                    nc.gpsimd.dma_start(wsb[:], wt_v[:, ot, :, :])
                    for tt in range(NT):
                        ts = slice(tt * TTILE, (tt + 1) * TTILE)
                        acc = ps_o.tile([P, TTILE], f32, tag="acc")
                        for ko in range(KO):
                            nc.tensor.matmul(
                                acc[:],
                                lhsT=wsb[:, ko, :],
                                rhs=xsb[:, ko, ts],
                                start=(ko == 0),
                                stop=False,
                            )
                        nc.tensor.matmul(
                            acc[:],
                            lhsT=wsb[:, KO, :],
                            rhs=hwsb[:, ts],
                            start=False,
                            stop=True,
                        )
                        osb = ob.tile([P, TTILE], f32, tag="osb")
                        nc.vector.tensor_tensor(
                            osb[:],
                            acc[:],
                            bbsb[:, ot, None].to_broadcast((P, TTILE)),
                            Alu.add,
                        )
                        nc.sync.dma_start(yt_v[:, ot, ts], osb[:])
            phase2.close()

    nc.compile()
    return nc


def get_program():
    if "nc" not in _prog_cache:
        _prog_cache["nc"] = _build_program()
    return _prog_cache["nc"]


def make_in_maps(x, base_w, base_b, lora_A, lora_B, router_w, scalings):
    """Host-side sharding/layout prep -> per-core input dicts."""
    import ml_dtypes

    wnp = ml_dtypes.bfloat16 if WT_BF16 else np.float32

    x = np.ascontiguousarray(x, dtype=np.float32)
    xt_full = np.ascontiguousarray(x.reshape(T, D).T)  # [D, T]

    lb_host = None
    if FP8_BASE:
        # base weights x64 -> e4m3, DoubleRow pair layout [ot,p,k2,j,f]
        wt_host = np.ascontiguousarray(
            (base_w.T.astype(np.float32) * W8_SCALE)
            .reshape(KO2, 2, P, OTILES, P)
            .transpose(3, 2, 0, 1, 4)
            .reshape(OTILES * P, KO2 * 2 * P)
            .astype(ml_dtypes.float8_e4m3)
        )
        lb_host = np.ascontiguousarray(lora_B.T.astype(np.float32).astype(wnp))
    else:
        # W_ext = [base_w.T ; lora_B.T]  ->  [ot, p, ko*128+f] layout
        w_ext = np.empty((KO_EXT * P, O), dtype=np.float32)
        w_ext[:D] = base_w.T
        w_ext[D:] = lora_B.T
        wt_host = np.ascontiguousarray(
            w_ext.reshape(KO_EXT, P, OTILES, P)
            .transpose(2, 1, 0, 3)
            .reshape(OTILES * P, KO_EXT * P)
            .astype(wnp)
        )

    # lora_A.T (unscaled; scaling folded into e8) -> [p, ko*128+r]
    ar_host = np.ascontiguousarray(
        lora_A.T.astype(np.float32)
        .reshape(KO, P, R)
        .transpose(1, 0, 2)
        .reshape(P, KO * R)
        .astype(wnp)
    )

    # router_w.T -> [p, ko*8+e], exact fp32
    rt_host = np.ascontiguousarray(
        router_w.T.astype(np.float32)
        .reshape(KO, P, E)
        .transpose(1, 0, 2)
        .reshape(P, KO * E)
    )

    # expert -> rank-slot expansion with per-expert scaling folded in;
    # when FP8_BASE the x64 weight scale is folded here too so the lora
    # matmul accumulates at the same scale as the fp8 base steps
    e8 = np.zeros((E, P), dtype=np.float32)
    s = np.asarray(scalings, dtype=np.float32)
    if FP8_BASE:
        s = s * W8_SCALE
    for e in range(E):
        e8[e, e * RANK : (e + 1) * RANK] = s[e]
    idm = np.eye(P, dtype=np.float32)
    bbf = base_b.astype(np.float32)

    xb_full = xt_full.astype(ml_dtypes.bfloat16)
    x8_full = xt_full.astype(ml_dtypes.float8_e4m3)

    in_maps = []
    for c in range(N_CORES):
        cs = slice(c * T_PC, (c + 1) * T_PC)
        m = {
            "xt": np.ascontiguousarray(xt_full[:, cs]),
            "xb": np.ascontiguousarray(xb_full[:, cs]),
            "x8": np.ascontiguousarray(x8_full[:, cs]),
            "wt": wt_host,
            "ar": ar_host,
            "rt": rt_host,
            "bb": bbf,
            "e8": e8,
            "idm": idm,
        }
        if FP8_BASE:
            m["lb"] = lb_host
        in_maps.append(m)
    return in_maps


def assemble_output(results):
    """Per-core yt [O, T_PC] -> full [B, S, O]."""
    yt_full = np.concatenate([r["yt"] for r in results], axis=1)  # [O, T]
    return np.ascontiguousarray(yt_full.T).reshape(B, S, O)


def kernel(**inputs):
    _ensure_path()
    from concourse.bass_utils import run_bass_kernel_spmd

    assert int(inputs["top_k"]) == 2
    nc = get_program()
    in_maps = make_in_maps(
        inputs["x"],
        inputs["base_w"],
        inputs["base_b"],
        inputs["lora_A"],
        inputs["lora_B"],
        inputs["router_w"],
        inputs["scalings"],
    )
    res = run_bass_kernel_spmd(nc, in_maps, list(range(N_CORES)))
    return assemble_output(res.results)


if __name__ == "__main__":
    # quick smoke: build the program only
    get_program()
    print("program built OK")


# revision 22
# speedup vs baseline: 1.1784x; 1.1784x over previous
"""Trainium2 Bass kernel for nn_GatedLinear (gated LoRA-MoE linear layer).

Math (see reference):
  base_out = x @ base_w.T + base_b
  logits   = x @ router_w.T ; top-2 softmax -> dense per-expert gate
  h        = x @ lora_A.T   ; rank_w = repeat(gate*scalings, 16)
  out      = base_out + (h * rank_w) @ lora_B.T

Sharding: pure data-parallel over batch*seq across 8 cores (1024 tokens
per core); all weights replicated. No collectives.

Device-side strategy (v5):
  * The host ships three copies of x.T: exact fp32 (router only --
    top-2 selection must match the fp32 reference bit-for-bit), bf16
    (h matmul), and fp8e4m3 (base matmul). fp32 chunks go through a
    small ring on two HWDGE queues; bf16/fp8 chunks DMA straight into
    resident tiles on the gpsimd queue. Sub-tile deps let every
    consumer start as soon as its ko-chunk lands.
  * Base matmul runs fp8e4m3 with perf_mode=DoubleRow: weights are
    host-scaled x64 into e4m3's range and packed [k2, 2, f] so each
    matmul contracts 256 deep -- half the instructions of bf16, 216ns
    per 256x128x512 step. The x64 scale is folded into the e8 gate
    expansion (so the lora step accumulates at the same scale) and
    removed in the bias epilogue (acc/64 + b, one DVE op).
  * lora_B.T stays resident bf16 and closes each PSUM accumulation
    group as a 33rd step with the gated rank activations; lora_A/h and
    the gating chain stay bf16/fp32, keeping total rel err ~1.1e-2
    (gate is 2e-2) with the fp8 noise confined to the base term.
  * Router matmuls (exact fp32) and h matmuls interleave across the
    ko-chunks so the PE stays busy while x streams in; gating runs
    token-major (PE transposes + DVE top-2 chain).
  * Weight DRAM layout is [ot, p, (k2 j f)] so each per-ot weight DMA
    is 128 contiguous runs (fast descriptor gen + full DMA bw).
  * DMA queues: fp32 x alternates sync/scalar; bf16+fp8 x and lora
    consts on gpsimd; weight stream on scalar; outputs on sync.

Output is produced transposed ([out_features, tokens] per core) and
de-transposed on the host.
"""

from contextlib import ExitStack

import numpy as np


def _ensure_path():
    try:
        import concourse.bass  # noqa: F401
    except ImportError:
        import sys

        for p in ("/opt/trn_rl_repo", "/root/.axon_site/_ro/trn_rl_repo"):
            if p not in sys.path:
                sys.path.insert(0, p)


N_CORES = 8
B, S, D, O = 4, 2048, 4096, 4096
T = B * S              # 8192 tokens total
T_PC = T // N_CORES    # 1024 tokens per core
E = 8                  # experts
RANK = 16
R = E * RANK           # 128 fused rank dim
P = 128
KO = D // P            # 32 k-subtiles of the contraction dim
KO_EXT = KO + 1        # +1 subtile holding lora_B.T
OTILES = O // P        # 32 output-feature tiles
TTILE = 512            # tokens per matmul moving operand
NT = T_PC // TTILE     # 2 token tiles per core
GT = 256               # gating token-tile size
NGT = T_PC // GT       # 4 gating tiles
NGC = GT // P          # 128-chunks per gating tile

WT_BF16 = True         # bf16 stationary weights (mixed with f32r moving)
FP8_BASE = True        # fp8e4m3 DoubleRow for the base matmul (2x PE rate)
KO2 = KO // 2          # paired k-subtiles for DoubleRow (256-deep contraction)
W8_SCALE = 64.0        # base_w std is 1/64; scale into e4m3's sweet spot

_prog_cache = {}


def _build_program():
    """Build the single-core SPMD Bass program (same on all 8 cores)."""
    _ensure_path()
    import concourse.bass as bass
    import concourse.mybir as mybir
    import concourse.tile as tile
    from concourse import bacc

    f32 = mybir.dt.float32
    f32r = mybir.dt.float32r
    bf16 = mybir.dt.bfloat16
    f8 = mybir.dt.float8e4
    wdt = bf16 if WT_BF16 else f32r
    Alu = mybir.AluOpType
    Act = mybir.ActivationFunctionType
    DR = mybir.MatmulPerfMode.DoubleRow

    nc = bacc.Bacc(
        "TRN2",
        target_bir_lowering=False,
        debug=False,
        num_devices=N_CORES,
    )

    xt = nc.dram_tensor("xt", [D, T_PC], f32, kind="ExternalInput").ap()
    xb = nc.dram_tensor("xb", [D, T_PC], bf16, kind="ExternalInput").ap()
    x8d = nc.dram_tensor("x8", [D, T_PC], f8, kind="ExternalInput").ap()
    xb_v = xb.rearrange("(ko p) t -> p ko t", p=P)
    x8_v = x8d.rearrange("(ko p) t -> p ko t", p=P)
    if FP8_BASE:
        # base weights only, x64-scaled fp8, DoubleRow pair layout
        wt = nc.dram_tensor(
            "wt", [OTILES * P, KO2 * 2 * P], f8, kind="ExternalInput"
        ).ap()
        wt_v = wt.rearrange("(ot p) (k j f) -> p ot k j f", p=P, j=2, f=P)
        lb = nc.dram_tensor("lb", [P, O], wdt, kind="ExternalInput").ap()
    else:
        wt = nc.dram_tensor(
            "wt", [OTILES * P, KO_EXT * P], wdt, kind="ExternalInput"
        ).ap()
        wt_v = wt.rearrange("(ot p) (ko f) -> p ot ko f", p=P, f=P)
        lb = None
    ar = nc.dram_tensor("ar", [P, KO * R], wdt, kind="ExternalInput").ap()
    rt = nc.dram_tensor("rt", [P, KO * E], f32, kind="ExternalInput").ap()
    bb = nc.dram_tensor("bb", [O], f32, kind="ExternalInput").ap()
    e8 = nc.dram_tensor("e8", [E, P], f32, kind="ExternalInput").ap()
    idm = nc.dram_tensor("idm", [P, P], f32, kind="ExternalInput").ap()
    yt = nc.dram_tensor("yt", [O, T_PC], f32, kind="ExternalOutput").ap()

    xt_v = xt.rearrange("(ko p) t -> p ko t", p=P)        # [128, 32, 1024]
    ar_v = ar.rearrange("p (ko r) -> p ko r", r=R)        # [128, 32, 128]
    rt_v = rt.rearrange("p (ko e) -> p ko e", e=E)        # [128, 32, 8]
    bb_v = bb.rearrange("(ot p) -> p ot", p=P)            # [128, 32]
    yt_v = yt.rearrange("(ot p) t -> p ot t", p=P)        # [128, 32, 1024]

    with tile.TileContext(nc) as tc:
        with (
            tc.tile_pool(name="perm", bufs=1) as pp,
            tc.tile_pool(name="obuf", bufs=3) as ob,
        ):
            # ---- small permanent constants (scalar/Activation queue) ----
            rsbf = pp.tile([P, KO, E], f32)
            nc.scalar.dma_start(rsbf[:], rt_v[:])
            bbsb = pp.tile([P, OTILES], f32)
            nc.scalar.dma_start(bbsb[:], bb_v[:])
            e8sb = pp.tile([E, P], f32)
            nc.scalar.dma_start(e8sb[:], e8[:])
            idsb = pp.tile([P, P], f32)
            nc.scalar.dma_start(idsb[:], idm[:])

            rgp = pp.tile([P, T_PC], f32)    # per-rank gates [r, t]
            hwsb = pp.tile([P, T_PC], wdt)   # gated rank activations [r, t]

            # resident bf16 copy of x (for h), filled chunk-by-chunk
            xsb = pp.tile([P, KO, T_PC], wdt)
            if FP8_BASE:
                # resident fp8 copy of x (for the base matmul)
                x8sb = pp.tile([P, KO, T_PC], f8)
                # lora_B.T resident bf16 (stationary of the lora matmul)
                lbsb = pp.tile([P, O], wdt)
                nc.gpsimd.dma_start(lbsb[:], lb[:])

            # lora_A.T (bf16 stationary), gpsimd queue
            arsb = pp.tile([P, KO, R], wdt)
            nc.gpsimd.dma_start(arsb[:], ar_v[:])

            # ---- phase 1: stream x; router logits (exact fp32) ----
            phase1 = ExitStack()
            xfp = phase1.enter_context(tc.tile_pool(name="xring", bufs=6))
            gp = phase1.enter_context(tc.tile_pool(name="gtmp", bufs=1))
            ps_l = phase1.enter_context(
                tc.tile_pool(name="ps_l", bufs=NGT, space="PSUM")
            )
            ps_b = phase1.enter_context(
                tc.tile_pool(name="ps_b", bufs=2, space="PSUM")
            )
            ps_h = phase1.enter_context(
                tc.tile_pool(name="ps_h", bufs=2, space="PSUM")
            )

            # interleave the 4 gating tiles' router matmuls and the h
            # matmuls per ko-chunk so the PE starts as soon as the first
            # x chunk lands; x alternates two DMA queues for bandwidth
            lgs_t = [
                ps_l.tile([E, GT], f32, tag="lg", name=f"lg{g}")
                for g in range(NGT)
            ]
            h_t = [
                ps_h.tile([P, TTILE], f32, tag="h", name=f"h{t}")
                for t in range(NT)
            ]
            for ko in range(KO):
                xc = xfp.tile([P, T_PC], f32, tag="xc")
                dma_q = nc.sync if ko % 2 == 0 else nc.scalar
                dma_q.dma_start(xc[:], xt_v[:, ko, :])
                nc.gpsimd.dma_start(xsb[:, ko, :], xb_v[:, ko, :])
                if FP8_BASE:
                    nc.gpsimd.dma_start(x8sb[:, ko, :], x8_v[:, ko, :])
                for g in range(NGT):
                    gs = slice(g * GT, (g + 1) * GT)
                    nc.tensor.matmul(
                        lgs_t[g][:],
                        lhsT=rsbf[:, ko, :],
                        rhs=xc[:, gs],
                        start=(ko == 0),
                        stop=(ko == KO - 1),
                    )
                for tt in range(NT):
                    ts = slice(tt * TTILE, (tt + 1) * TTILE)
                    nc.tensor.matmul(
                        h_t[tt][:],
                        lhsT=arsb[:, ko, :],
                        rhs=xsb[:, ko, ts],
                        start=(ko == 0),
                        stop=(ko == KO - 1),
                    )

            for g in range(NGT):
                gs = slice(g * GT, (g + 1) * GT)
                lgs = gp.tile([E, GT], f32, tag="lgs")
                nc.vector.tensor_copy(lgs[:], lgs_t[g][:])

                # transpose logits to token-major: [tok, chunk, expert]
                ltk = gp.tile([P, NGC, E], f32, tag="ltk")
                for c in range(NGC):
                    tp = ps_b.tile([P, GT], f32, tag="pb", name="tp")[:, :E]
                    nc.tensor.transpose(
                        tp[:], lgs[:, c * P : (c + 1) * P], idsb[:E, :E]
                    )
                    nc.vector.tensor_copy(ltk[:, c, :], tp[:])

                # top-2 + softmax along the free (expert) axis.
                m1 = gp.tile([P, NGC, 1], f32, tag="m1")
                nc.vector.tensor_reduce(m1[:], ltk[:], mybir.AxisListType.X, Alu.max)
                mask1 = gp.tile([P, NGC, E], f32, tag="mask1")
                nc.vector.tensor_tensor(
                    mask1[:], ltk[:], m1.to_broadcast((P, NGC, E)), Alu.is_equal
                )
                l2 = gp.tile([P, NGC, E], f32, tag="l2")
                nc.vector.scalar_tensor_tensor(
                    l2[:], mask1[:], -1e30, ltk[:], Alu.mult, Alu.add
                )
                m2 = gp.tile([P, NGC, 1], f32, tag="m2")
                nc.vector.tensor_reduce(m2[:], l2[:], mybir.AxisListType.X, Alu.max)
                mask2 = gp.tile([P, NGC, E], f32, tag="mask2")
                nc.vector.tensor_tensor(
                    mask2[:], l2[:], m2.to_broadcast((P, NGC, E)), Alu.is_equal
                )
                dlt = gp.tile([P, NGC, 1], f32, tag="dlt")
                nc.vector.tensor_tensor(dlt[:], m2[:], m1[:], Alu.subtract)
                g2 = gp.tile([P, NGC, 1], f32, tag="g2")
                nc.scalar.activation(g2[:], dlt[:], Act.Sigmoid)
                g1 = gp.tile([P, NGC, 1], f32, tag="g1")
                nc.vector.tensor_scalar(g1[:], g2[:], -1.0, 1.0, Alu.mult, Alu.add)

                gate = gp.tile([P, NGC, E], f32, tag="gate")
                nc.vector.tensor_tensor(
                    gate[:], mask1[:], g1.to_broadcast((P, NGC, E)), Alu.mult
                )
                gm2 = gp.tile([P, NGC, E], f32, tag="gm2")
                nc.vector.tensor_tensor(
                    gm2[:], mask2[:], g2.to_broadcast((P, NGC, E)), Alu.mult
                )
                nc.vector.tensor_tensor(gate[:], gate[:], gm2[:], Alu.add)

                # transpose gates back to expert-major [8, 256]
                gts = gp.tile([E, GT], f32, tag="gts")
                for c in range(NGC):
                    tp2 = ps_b.tile([P, GT], f32, tag="pb", name="tp2")[:E, :P]
                    nc.tensor.transpose(tp2[:], gate[:, c, :], idsb[:])
                    nc.vector.tensor_copy(gts[:, c * P : (c + 1) * P], tp2[:])

                # expand expert gates (x scaling, folded into e8) to the
                # 128 rank slots: RG = e8.T @ gts
                RG = ps_b.tile([P, GT], f32, tag="pb", name="RG")
                nc.tensor.matmul(
                    RG[:], lhsT=e8sb[:], rhs=gts[:], start=True, stop=True
                )
                nc.vector.tensor_copy(rgp[:, gs], RG[:])

            # gated rank activations (x64-scaled when FP8_BASE: the x64 is
            # folded into e8 -> rgp, cancelling the fp8 weight scale)
            for tt in range(NT):
                ts = slice(tt * TTILE, (tt + 1) * TTILE)
                nc.vector.tensor_tensor(
                    hwsb[:, ts], h_t[tt][:], rgp[:, ts], Alu.mult
                )

            phase1.close()

            # ---- phase 2: base matmul + fused lora_B ----
            phase2 = ExitStack()
            ps_o = phase2.enter_context(
                tc.tile_pool(name="ps_o", bufs=5, space="PSUM")
            )
            wpool = phase2.enter_context(tc.tile_pool(name="wstream", bufs=4))

            for ot in range(OTILES):
                os_ = slice(ot * P, (ot + 1) * P)
                if FP8_BASE:
                    wsb = wpool.tile([P, KO2, 2, P], f8, tag="w")
                    nc.gpsimd.dma_start(wsb[:], wt_v[:, ot, :, :, :])
                    for tt in range(NT):
                        ts = slice(tt * TTILE, (tt + 1) * TTILE)
                        acc = ps_o.tile([P, TTILE], f32, tag="acc")
                        for k2 in range(KO2):
                            nc.tensor.matmul(
                                acc[:],
                                lhsT=wsb[:, k2, :, :],
                                rhs=x8sb[:, 2 * k2 : 2 * k2 + 2, ts],
                                start=(k2 == 0),
                                stop=False,
                                perf_mode=DR,
                            )
                        nc.tensor.matmul(
                            acc[:],
                            lhsT=lbsb[:, os_],
                            rhs=hwsb[:, ts],
                            start=False,
                            stop=True,
                        )
                        osb = ob.tile([P, TTILE], f32, tag="osb")
                        # acc holds 64x(base+lora); rescale + bias in one op
                        nc.vector.scalar_tensor_tensor(
                            osb[:],
                            acc[:],
                            1.0 / W8_SCALE,
                            bbsb[:, ot, None].to_broadcast((P, TTILE)),
                            Alu.mult,
                            Alu.add,
                        )
                        nc.sync.dma_start(yt_v[:, ot, ts], osb[:])
                else:
                    wsb = wpool.tile([P, KO_EXT, P], wdt, tag="w")
                    nc.gpsimd.dma_start(wsb[:], wt_v[:, ot, :, :])
                    for tt in range(NT):
                        ts = slice(tt * TTILE, (tt + 1) * TTILE)
                        acc = ps_o.tile([P, TTILE], f32, tag="acc")
                        for ko in range(KO):
                            nc.tensor.matmul(
                                acc[:],
                                lhsT=wsb[:, ko, :],
                                rhs=xsb[:, ko, ts],
                                start=(ko == 0),
                                stop=False,
                            )
                        nc.tensor.matmul(
                            acc[:],
                            lhsT=wsb[:, KO, :],
                            rhs=hwsb[:, ts],
                            start=False,
                            stop=True,
                        )
                        osb = ob.tile([P, TTILE], f32, tag="osb")
                        nc.vector.tensor_tensor(
                            osb[:],
                            acc[:],
                            bbsb[:, ot, None].to_broadcast((P, TTILE)),
                            Alu.add,
                        )
                        nc.sync.dma_start(yt_v[:, ot, ts], osb[:])
            phase2.close()

    nc.compile()
    return nc


def get_program():
    if "nc" not in _prog_cache:
        _prog_cache["nc"] = _build_program()
    return _prog_cache["nc"]


def make_in_maps(x, base_w, base_b, lora_A, lora_B, router_w, scalings):
    """Host-side sharding/layout prep -> per-core input dicts."""
    import ml_dtypes

    wnp = ml_dtypes.bfloat16 if WT_BF16 else np.float32

    x = np.ascontiguousarray(x, dtype=np.float32)
    xt_full = np.ascontiguousarray(x.reshape(T, D).T)  # [D, T]

    lb_host = None
    if FP8_BASE:
        # base weights x64 -> e4m3, DoubleRow pair layout [ot,p,k2,j,f]
        wt_host = np.ascontiguousarray(
            (base_w.T.astype(np.float32) * W8_SCALE)
            .reshape(KO2, 2, P, OTILES, P)
            .transpose(3, 2, 0, 1, 4)
            .reshape(OTILES * P, KO2 * 2 * P)
            .astype(ml_dtypes.float8_e4m3)
        )
        lb_host = np.ascontiguousarray(lora_B.T.astype(np.float32).astype(wnp))
    else:
        # W_ext = [base_w.T ; lora_B.T]  ->  [ot, p, ko*128+f] layout
        w_ext = np.empty((KO_EXT * P, O), dtype=np.float32)
        w_ext[:D] = base_w.T
        w_ext[D:] = lora_B.T
        wt_host = np.ascontiguousarray(
            w_ext.reshape(KO_EXT, P, OTILES, P)
            .transpose(2, 1, 0, 3)
            .reshape(OTILES * P, KO_EXT * P)
            .astype(wnp)
        )

    # lora_A.T (unscaled; scaling folded into e8) -> [p, ko*128+r]
    ar_host = np.ascontiguousarray(
        lora_A.T.astype(np.float32)
        .reshape(KO, P, R)
        .transpose(1, 0, 2)
        .reshape(P, KO * R)
        .astype(wnp)
    )

    # router_w.T -> [p, ko*8+e], exact fp32
    rt_host = np.ascontiguousarray(
        router_w.T.astype(np.float32)
        .reshape(KO, P, E)
        .transpose(1, 0, 2)
        .reshape(P, KO * E)
    )

    # expert -> rank-slot expansion with per-expert scaling folded in;
    # when FP8_BASE the x64 weight scale is folded here too so the lora
    # matmul accumulates at the same scale as the fp8 base steps
    e8 = np.zeros((E, P), dtype=np.float32)
    s = np.asarray(scalings, dtype=np.float32)
    if FP8_BASE:
        s = s * W8_SCALE
    for e in range(E):
        e8[e, e * RANK : (e + 1) * RANK] = s[e]
    idm = np.eye(P, dtype=np.float32)
    bbf = base_b.astype(np.float32)

    xb_full = xt_full.astype(ml_dtypes.bfloat16)
    x8_full = xt_full.astype(ml_dtypes.float8_e4m3)

    in_maps = []
    for c in range(N_CORES):
        cs = slice(c * T_PC, (c + 1) * T_PC)
        m = {
            "xt": np.ascontiguousarray(xt_full[:, cs]),
            "xb": np.ascontiguousarray(xb_full[:, cs]),
            "x8": np.ascontiguousarray(x8_full[:, cs]),
            "wt": wt_host,
            "ar": ar_host,
            "rt": rt_host,
            "bb": bbf,
            "e8": e8,
            "idm": idm,
        }
        if FP8_BASE:
            m["lb"] = lb_host
        in_maps.append(m)
    return in_maps


def assemble_output(results):
    """Per-core yt [O, T_PC] -> full [B, S, O]."""
    yt_full = np.concatenate([r["yt"] for r in results], axis=1)  # [O, T]
    return np.ascontiguousarray(yt_full.T).reshape(B, S, O)


def kernel(**inputs):
    _ensure_path()
    from concourse.bass_utils import run_bass_kernel_spmd

    assert int(inputs["top_k"]) == 2
    nc = get_program()
    in_maps = make_in_maps(
        inputs["x"],
        inputs["base_w"],
        inputs["base_b"],
        inputs["lora_A"],
        inputs["lora_B"],
        inputs["router_w"],
        inputs["scalings"],
    )
    res = run_bass_kernel_spmd(nc, in_maps, list(range(N_CORES)))
    return assemble_output(res.results)


if __name__ == "__main__":
    # quick smoke: build the program only
    get_program()
    print("program built OK")
